# revision 1
# baseline (speedup 1.0000x reference)
"""LlamaAttention (B=1, S=2048, D=2048, H=16, KV=4) on 8 TRN2 NeuronCores.

Tensor-parallel over heads: core c owns q-heads [2c, 2c+1] and kv-head c//2.
Each core computes partial = attn_out_c @ Wo[:, c-slice].T over the full
sequence; the all-reduce after o_proj happens on the host (sum of partials).

Layout strategy: everything on-chip lives feature-on-partitions ("transposed"):
  hsT [d, s], qT/kT/vT [hd, s], attn_outT [hd, s].  The host pre-transposes
hidden_states and weights into partition-major [128, N] arrays so every DMA is
contiguous.  RoPE tables (cos / sign-adjusted sin), the causal diagonal mask
block, and the bf16 identity (for PE transposes) are precomputed on host.

Matmuls run as float32r (TF32-grade, 1 cycle/row at N>=256); P/V in attention
run bf16.  Softmax skips the running-max (scores are O(6) sigma, exp cannot
overflow fp32) and uses the scalar engine's accum_out for the row sums.
"""
import math
import numpy as np

S = 2048
D = 2048
HD = 128
H = 16
KV = 4
NCORES = 8
NT = S // 128          # 16 sequence tiles
DTC = D // 128         # 16 feature chunks
QH = H // NCORES       # 2 q-heads per core
ROPE_BASE = 10000.0
SCALE = 1.0 / math.sqrt(HD)
NEG = -1.0e9

_CACHE = {}


def _rope(nc, pool, dst, src_ps, cos_sb, sin_sb, cols, F32, ALU):
    """dst[:, cols] = src*cos + rotate_half(src)*sin  (src: psum [128, w])."""
    w = cols.stop - cols.start
    tmp = pool.tile([128, w], F32, tag="ropetmp")
    nc.scalar.copy(out=tmp[0:64, :], in_=src_ps[64:128, :])
    nc.scalar.copy(out=tmp[64:128, :], in_=src_ps[0:64, :])
    nc.vector.tensor_tensor(out=dst[:, cols], in0=src_ps, in1=cos_sb[:, cols], op=ALU.mult)
    nc.vector.tensor_tensor(out=tmp, in0=tmp, in1=sin_sb[:, cols], op=ALU.mult)
    nc.vector.tensor_tensor(out=dst[:, cols], in0=dst[:, cols], in1=tmp, op=ALU.add)


def build_nc():
    import concourse.bacc as bacc
    import concourse.tile as tile
    from concourse import mybir

    F32 = mybir.dt.float32
    F32R = mybir.dt.float32r
    BF16 = mybir.dt.bfloat16
    AF = mybir.ActivationFunctionType
    ALU = mybir.AluOpType

    nc = bacc.Bacc("TRN2", target_bir_lowering=False, debug=False)
    hs_d = nc.dram_tensor("hs", [128, DTC * S], F32R, kind="ExternalInput").ap()
    wq_d = nc.dram_tensor("wq", [128, DTC * QH * 128], F32R, kind="ExternalInput").ap()
    wk_d = nc.dram_tensor("wk", [128, DTC * 128], F32R, kind="ExternalInput").ap()
    wv_d = nc.dram_tensor("wv", [128, DTC * 128], F32R, kind="ExternalInput").ap()
    wo_d = nc.dram_tensor("wo", [128, QH * D], F32R, kind="ExternalInput").ap()
    cos_d = nc.dram_tensor("cos", [128, S], F32, kind="ExternalInput").ap()
    sin_d = nc.dram_tensor("sin", [128, S], F32, kind="ExternalInput").ap()
    tri_d = nc.dram_tensor("tri", [128, 128], F32, kind="ExternalInput").ap()
    id_d = nc.dram_tensor("ident", [128, 128], BF16, kind="ExternalInput").ap()
    out_d = nc.dram_tensor("out", [128, NT * D], F32, kind="ExternalOutput").ap()

    hs3 = hs_d.rearrange("p (t s) -> p t s", t=DTC)
    out3 = out_d.rearrange("p (t d) -> p t d", t=NT)

    HS_HALF = S // 2
    NG = NT // 4

    with tile.TileContext(nc) as tc:
        with tc.tile_pool(name="consts", bufs=1) as consts, \
             tc.tile_pool(name="persist", bufs=1) as persist, \
             tc.tile_pool(name="stats", bufs=1) as stats:
            cos_sb = consts.tile([128, S], F32)
            sin_sb = consts.tile([128, S], F32)
            tri_sb = consts.tile([128, 128], F32)
            id_sb = consts.tile([128, 128], BF16)
            wq_sb = consts.tile([128, DTC, QH * 128], F32R)
            wk_sb = consts.tile([128, DTC, 128], F32R)
            wv_sb = consts.tile([128, DTC, 128], F32R)
            wo_sb = consts.tile([128, QH, D], F32R)
            nc.sync.dma_start(out=cos_sb, in_=cos_d)
            nc.sync.dma_start(out=sin_sb, in_=sin_d)
            nc.sync.dma_start(out=tri_sb, in_=tri_d)
            nc.sync.dma_start(out=id_sb, in_=id_d)
            nc.sync.dma_start(out=wq_sb, in_=wq_d.rearrange("p (t m) -> p t m", t=DTC))
            nc.sync.dma_start(out=wk_sb, in_=wk_d.rearrange("p (t m) -> p t m", t=DTC))
            nc.sync.dma_start(out=wv_sb, in_=wv_d.rearrange("p (t m) -> p t m", t=DTC))
            nc.sync.dma_start(out=wo_sb, in_=wo_d.rearrange("p (h m) -> p h m", h=QH))

            qrot = [persist.tile([128, S], F32R, tag=f"qrot{h}", name=f"qrot{h}") for h in range(QH)]
            krot = persist.tile([128, S], F32R, tag="krot")
            vbf = persist.tile([128, S], BF16, tag="vbf")
            vnat = persist.tile([128, NT * 128], BF16, tag="vnat")
            aout = [persist.tile([128, S], F32R, tag=f"aout{h}", name=f"aout{h}") for h in range(QH)]
            l_sb = stats.tile([128, QH * NT], F32, tag="l")
            linv_sb = stats.tile([128, QH * NT], F32, tag="linv")

            # ---------------- QKV projections (+RoPE), s-half at a time -----
            with tc.tile_pool(name="hsp", bufs=2) as hsp, \
                 tc.tile_pool(name="ropet", bufs=2) as ropet, \
                 tc.tile_pool(name="qkvps", bufs=1, space="PSUM") as qkvps:
                for sh in range(2):
                    cols = slice(sh * HS_HALF, (sh + 1) * HS_HALF)
                    pq = [qkvps.tile([128, HS_HALF], F32, tag=f"pq{m}", name=f"pq{m}") for m in range(QH)]
                    pk = qkvps.tile([128, HS_HALF], F32, tag="pk")
                    pv = qkvps.tile([128, HS_HALF], F32, tag="pv")
                    for j in range(DTC // 2):
                        hst = hsp.tile([128, 2, HS_HALF], F32R, tag="hst")
                        nc.sync.dma_start(
                            out=hst,
                            in_=hs3[:, 2 * j:2 * j + 2, sh * HS_HALF:(sh + 1) * HS_HALF])
                        for t2 in range(2):
                            dt = 2 * j + t2
                            st = dt == 0
                            sp = dt == DTC - 1
                            for n in range(HS_HALF // 512):
                                ns = slice(n * 512, (n + 1) * 512)
                                rhs = hst[:, t2, ns]
                                for m in range(QH):
                                    nc.tensor.matmul(pq[m][:, ns],
                                                     wq_sb[:, dt, m * 128:(m + 1) * 128],
                                                     rhs, start=st, stop=sp)
                                nc.tensor.matmul(pk[:, ns], wk_sb[:, dt, :], rhs,
                                                 start=st, stop=sp)
                                nc.tensor.matmul(pv[:, ns], wv_sb[:, dt, :], rhs,
                                                 start=st, stop=sp)
                    for m in range(QH):
                        _rope(nc, ropet, qrot[m], pq[m], cos_sb, sin_sb, cols, F32, ALU)
                    _rope(nc, ropet, krot, pk, cos_sb, sin_sb, cols, F32, ALU)
                    nc.vector.tensor_copy(out=vbf[:, cols], in_=pv)

            # ---------------- attention --------------------------------------
            with tc.tile_pool(name="sps", bufs=1, space="PSUM") as sps, \
                 tc.tile_pool(name="ptps", bufs=2, space="PSUM") as ptps, \
                 tc.tile_pool(name="pvps", bufs=1, space="PSUM") as pvps, \
                 tc.tile_pool(name="pp", bufs=5) as pp, \
                 tc.tile_pool(name="pts", bufs=3) as pts:
                # v: [hd, s] -> natural [s, hd] blocks via PE transpose
                for t4 in range(NT // 4):
                    vt = ptps.tile([128, 512], BF16, tag="pt")
                    for ii in range(4):
                        t = t4 * 4 + ii
                        nc.tensor.transpose(vt[:, ii * 128:(ii + 1) * 128],
                                            vbf[:, t * 128:(t + 1) * 128], id_sb)
                    nc.vector.tensor_copy(out=vnat[:, t4 * 512:(t4 + 1) * 512], in_=vt)

                for g in range(NG):
                    for h in range(QH):
                        ptiles = []
                        for ii in range(4):
                            i = 4 * g + ii
                            width = (i + 1) * 128
                            s_ps = sps.tile([128, S], F32, tag="s")
                            for c0 in range(0, width, 512):
                                ce = min(c0 + 512, width)
                                nc.tensor.matmul(s_ps[:, c0:ce],
                                                 qrot[h][:, i * 128:(i + 1) * 128],
                                                 krot[:, c0:ce], start=True, stop=True)
                            nc.vector.tensor_tensor(out=s_ps[:, i * 128:width],
                                                    in0=s_ps[:, i * 128:width],
                                                    in1=tri_sb, op=ALU.add)
                            p_i = pp.tile([128, S], BF16, tag="p")
                            col = h * NT + i
                            nc.scalar.activation(out=p_i[:, 0:width], in_=s_ps[:, 0:width],
                                                 func=AF.Exp, scale=SCALE,
                                                 accum_out=l_sb[:, col:col + 1])
                            nc.vector.reciprocal(out=linv_sb[:, col:col + 1],
                                                 in_=l_sb[:, col:col + 1])
                            nc.vector.tensor_scalar_mul(p_i[:, 0:width], p_i[:, 0:width],
                                                        linv_sb[:, col:col + 1])
                            ptiles.append((i, width, p_i))
                        pv_ps = pvps.tile([128, 512], F32, tag="pvacc")
                        jmax = 4 * g + 3
                        for j in range(jmax + 1):
                            ii_lo = max(0, j - 4 * g)
                            pt_ps = ptps.tile([128, 512], BF16, tag="pt")
                            for ii in range(ii_lo, 4):
                                i, width, p_i = ptiles[ii]
                                nc.tensor.transpose(pt_ps[:, ii * 128:(ii + 1) * 128],
                                                    p_i[:, j * 128:(j + 1) * 128], id_sb)
                            pt_sb = pts.tile([128, 512], BF16, tag="ptsb")
                            nc.vector.tensor_copy(out=pt_sb[:, ii_lo * 128:512],
                                                  in_=pt_ps[:, ii_lo * 128:512])
                            nc.tensor.matmul(pv_ps[:, ii_lo * 128:512],
                                             vnat[:, j * 128:(j + 1) * 128],
                                             pt_sb[:, ii_lo * 128:512],
                                             start=(j == 0), stop=(j == jmax))
                        nc.vector.tensor_copy(out=aout[h][:, g * 512:(g + 1) * 512],
                                              in_=pv_ps)

            # ---------------- o_proj -----------------------------------------
            with tc.tile_pool(name="ops", bufs=4, space="PSUM") as ops, \
                 tc.tile_pool(name="osb", bufs=2) as osb:
                for t in range(NT):
                    o_sb = osb.tile([128, D], F32, tag="osb")
                    for n in range(D // 512):
                        po = ops.tile([128, 512], F32, tag="po")
                        for h in range(QH):
                            nc.tensor.matmul(po, aout[h][:, t * 128:(t + 1) * 128],
                                             wo_sb[:, h, n * 512:(n + 1) * 512],
                                             start=(h == 0), stop=(h == QH - 1))
                        nc.scalar.copy(out=o_sb[:, n * 512:(n + 1) * 512], in_=po)
                    nc.sync.dma_start(out=out3[:, t, :], in_=o_sb)

    nc.compile()
    return nc


def _pm(x):
    """[n*128, M] row-major -> partition-major [128, n*M]."""
    n = x.shape[0] // 128
    return np.ascontiguousarray(
        x.reshape(n, 128, x.shape[1]).transpose(1, 0, 2).reshape(128, -1))


def prep_in_maps(hidden_states, position_ids, Wq, Wk, Wv, Wo):
    import ml_dtypes
    hs = np.asarray(hidden_states, dtype=np.float32).reshape(S, D)
    hsT_pm = _pm(np.ascontiguousarray(hs.T))                       # [128, DTC*S]

    pos = np.asarray(position_ids).reshape(S).astype(np.float32)
    inv = (ROPE_BASE ** (-np.arange(0, HD, 2, dtype=np.float32) / HD))  # [64]
    ang = np.concatenate([pos[None, :] * inv[:, None]] * 2, axis=0)     # [128, S]
    cos_t = np.cos(ang).astype(np.float32)
    sin_t = np.sin(ang).astype(np.float32)
    sin_signed = np.concatenate([-sin_t[:64], sin_t[64:]], axis=0)

    q_idx = np.arange(128)[:, None]
    k_idx = np.arange(128)[None, :]
    tri = np.where(k_idx <= q_idx, 0.0, NEG).astype(np.float32)
    ident = np.eye(128, dtype=ml_dtypes.bfloat16)

    Wq = np.asarray(Wq, np.float32)
    Wk = np.asarray(Wk, np.float32)
    Wv = np.asarray(Wv, np.float32)
    Wo = np.asarray(Wo, np.float32)

    in_maps = []
    for c in range(NCORES):
        g = (c * QH) // (H // KV)          # kv head owned by this core
        wq_c = Wq[c * QH * 128:(c + 1) * QH * 128]      # [256, D]
        wk_c = Wk[g * 128:(g + 1) * 128]                # [128, D]
        wv_c = Wv[g * 128:(g + 1) * 128]                # [128, D]
        wo_c = Wo[:, c * QH * 128:(c + 1) * QH * 128]   # [D, 256]
        in_maps.append({
            "hs": hsT_pm,
            "wq": _pm(np.ascontiguousarray(wq_c.T)),
            "wk": _pm(np.ascontiguousarray(wk_c.T)),
            "wv": _pm(np.ascontiguousarray(wv_c.T)),
            "wo": _pm(np.ascontiguousarray(wo_c.T)),
            "cos": cos_t,
            "sin": sin_signed,
            "tri": tri,
            "ident": ident,
        })
    return in_maps


def combine_outputs(results):
    total = np.zeros((S, D), np.float32)
    for r in results:
        o = np.asarray(r["out"], np.float32)
        total += o.reshape(128, NT, D).transpose(1, 0, 2).reshape(S, D)
    return total[None]


def kernel(hidden_states, attention_mask, position_ids, Wq, Wk, Wv, Wo):
    from concourse import bass_utils
    if "nc" not in _CACHE:
        _CACHE["nc"] = build_nc()
    nc = _CACHE["nc"]
    in_maps = prep_in_maps(hidden_states, position_ids, Wq, Wk, Wv, Wo)
    res = bass_utils.run_bass_kernel_spmd(nc, in_maps, core_ids=list(range(NCORES)))
    return combine_outputs(res.results)



# revision 2
# speedup vs baseline: 1.5470x; 1.5470x over previous
"""LlamaAttention (B=1, S=2048, D=2048, H=16, KV=4) on 8 TRN2 NeuronCores.

Tensor-parallel over heads: core c owns q-heads [2c, 2c+1] and kv-head c//2.
Each core computes partial = attn_out_c @ Wo[:, c-slice].T over the full
sequence; the all-reduce after o_proj happens on the host (sum of partials).

v2 design (vs the fp32r baseline):
  * QKV projections run in fp16 (hs + Wq/Wk/Wv fp16): halves the input DMA
    and matches fp32r accuracy (10 vs 11 mantissa bits).  Quarter-sized
    (512-col) PSUM accumulation with chunked, double-buffered hs DMAs so the
    first matmul starts ~2us in.
  * Attention is computed TRANSPOSED: sT[k,q] = krot_j^T qrot (bf16), mask
    on the diagonal block only, exp -> E^T tiles, PV via lhsT=vnat (natural
    [s,hd] V) streaming E^T -> aout[hd,q].  No per-block P transposes at all.
  * Softmax row sums l[q]: E^T tiles are accumulated into racc[k%128, q] on
    the (otherwise idle) Pool engine, then per 128-col tile: PE transpose +
    DVE free-dim reduce -> l column; reciprocal -> linv.
  * Normalization rides a double PE-transpose of aout (bf16, cheap): the
    intermediate [q, hd] orientation puts q on partitions, so the 1/l scale
    folds into the PSUM->SBUF copy (scalar activation, per-partition scale).
  * o_proj (bf16) is interleaved per 512-col chunk with attention; output
    partials are written bf16 ([128, t, D] per q-tile), summed on host.
"""
import math
import numpy as np

S = 2048
D = 2048
HD = 128
H = 16
KV = 4
NCORES = 8
NT = S // 128           # 16 sequence tiles
DTC = D // 128          # 16 feature chunks
NQ = 4                  # 512-col sequence quarters
QC = S // NQ            # 512
QH = H // NCORES        # 2 q-heads per core
ROPE_BASE = 10000.0
SCALE = 1.0 / math.sqrt(HD)
NEG = -1.0e9

_CACHE = {}


def _rope(nc, pool, dst, cols, src_ps, cos_sb, sin_sb, F32, ALU):
    """dst[:, cols] = src*cos + rotate_half(src)*sin  (src: psum [128, 512])."""
    w = cols.stop - cols.start
    tmp = pool.tile([128, w], F32, tag="ropetmp")
    nc.scalar.copy(out=tmp[0:64, :], in_=src_ps[64:128, :])
    nc.scalar.copy(out=tmp[64:128, :], in_=src_ps[0:64, :])
    nc.vector.tensor_tensor(out=dst[:, cols], in0=src_ps, in1=cos_sb[:, cols], op=ALU.mult)
    nc.vector.tensor_tensor(out=tmp, in0=tmp, in1=sin_sb[:, cols], op=ALU.mult)
    nc.vector.tensor_tensor(out=dst[:, cols], in0=dst[:, cols], in1=tmp, op=ALU.add)


def build_nc():
    import concourse.bacc as bacc
    import concourse.tile as tile
    from concourse import mybir

    F32 = mybir.dt.float32
    F16 = mybir.dt.float16
    BF16 = mybir.dt.bfloat16
    AF = mybir.ActivationFunctionType
    ALU = mybir.AluOpType
    AX = mybir.AxisListType

    nc = bacc.Bacc("TRN2", target_bir_lowering=False, debug=False)
    hs_d = nc.dram_tensor("hs", [128, NQ * DTC * QC], F16, kind="ExternalInput").ap()
    wq_d = nc.dram_tensor("wq", [128, DTC * QH * 128], F16, kind="ExternalInput").ap()
    wk_d = nc.dram_tensor("wk", [128, DTC * 128], F16, kind="ExternalInput").ap()
    wv_d = nc.dram_tensor("wv", [128, DTC * 128], F16, kind="ExternalInput").ap()
    wo_d = nc.dram_tensor("wo", [128, QH * D], BF16, kind="ExternalInput").ap()
    cos_d = nc.dram_tensor("cos", [128, S], F32, kind="ExternalInput").ap()
    sin_d = nc.dram_tensor("sin", [128, S], F32, kind="ExternalInput").ap()
    trit_d = nc.dram_tensor("trit", [128, 128], F32, kind="ExternalInput").ap()
    idb_d = nc.dram_tensor("idb", [128, 128], BF16, kind="ExternalInput").ap()
    idf_d = nc.dram_tensor("idf", [128, 128], F32, kind="ExternalInput").ap()
    out_d = nc.dram_tensor("out", [128, NT * D], BF16, kind="ExternalOutput").ap()

    hs4 = hs_d.rearrange("p (q t s) -> p q t s", q=NQ, t=DTC)
    wq3 = wq_d.rearrange("p (t m) -> p t m", t=DTC)
    wk3 = wk_d.rearrange("p (t m) -> p t m", t=DTC)
    wv3 = wv_d.rearrange("p (t m) -> p t m", t=DTC)
    out3 = out_d.rearrange("p (t d) -> p t d", t=NT)

    with tile.TileContext(nc) as tc:
        with tc.tile_pool(name="consts", bufs=1) as consts, \
             tc.tile_pool(name="persist", bufs=1) as persist:
            cos_sb = consts.tile([128, S], F32)
            sin_sb = consts.tile([128, S], F32)
            trit_sb = consts.tile([128, 128], F32)
            idb_sb = consts.tile([128, 128], BF16)
            idf_sb = consts.tile([128, 128], F32)
            wq_sb = consts.tile([128, DTC, QH * 128], F16)
            wk_sb = consts.tile([128, DTC, 128], F16)
            wv_sb = consts.tile([128, DTC, 128], F16)
            wo_sb = consts.tile([128, QH, D], BF16)

            # weights in 4-dt-group slices so the first matmuls start early
            for g4 in range(4):
                ts = slice(g4 * 4, (g4 + 1) * 4)
                nc.sync.dma_start(out=wq_sb[:, ts, :], in_=wq3[:, ts, :])
                nc.sync.dma_start(out=wk_sb[:, ts, :], in_=wk3[:, ts, :])
                nc.sync.dma_start(out=wv_sb[:, ts, :], in_=wv3[:, ts, :])
            nc.sync.dma_start(out=cos_sb, in_=cos_d)
            nc.sync.dma_start(out=sin_sb, in_=sin_d)
            nc.sync.dma_start(out=trit_sb, in_=trit_d)
            nc.sync.dma_start(out=idb_sb, in_=idb_d)
            nc.sync.dma_start(out=idf_sb, in_=idf_d)
            nc.sync.dma_start(out=wo_sb, in_=wo_d.rearrange("p (h m) -> p h m", h=QH))

            qrot = [persist.tile([128, S], BF16, tag=f"qrot{h}", name=f"qrot{h}")
                    for h in range(QH)]
            krot = persist.tile([128, S], BF16, tag="krot")
            vbf = persist.tile([128, S], BF16, tag="vbf")
            vnat = persist.tile([128, NT * 128], BF16, tag="vnat")
            afin = [persist.tile([128, S], BF16, tag=f"afin{h}", name=f"afin{h}")
                    for h in range(QH)]
            racc = [persist.tile([128, S], F32, tag=f"racc{h}", name=f"racc{h}")
                    for h in range(QH)]
            linv_sb = persist.tile([128, QH * NT], F32, tag="linv")
            l_sb = persist.tile([128, QH * NT], F32, tag="l")

            # ---------------- QKV projections (+RoPE), 512-col quarters ------
            with tc.tile_pool(name="hsp", bufs=3) as hsp, \
                 tc.tile_pool(name="ropet", bufs=2) as ropet, \
                 tc.tile_pool(name="qkvps", bufs=2, space="PSUM") as qkvps:
                for sq in range(NQ):
                    cols = slice(sq * QC, (sq + 1) * QC)
                    pq = [qkvps.tile([128, QC], F32, tag=f"pq{m}", name=f"pq{m}")
                          for m in range(QH)]
                    pk = qkvps.tile([128, QC], F32, tag="pk")
                    pv = qkvps.tile([128, QC], F32, tag="pv")
                    for g4 in range(4):
                        hst = hsp.tile([128, 4, QC], F16, tag="hst")
                        nc.sync.dma_start(out=hst,
                                          in_=hs4[:, sq, g4 * 4:(g4 + 1) * 4, :])
                        for t4 in range(4):
                            dt = g4 * 4 + t4
                            st = dt == 0
                            sp = dt == DTC - 1
                            rhs = hst[:, t4, :]
                            for m in range(QH):
                                nc.tensor.matmul(pq[m], wq_sb[:, dt, m * 128:(m + 1) * 128],
                                                 rhs, start=st, stop=sp)
                            nc.tensor.matmul(pk, wk_sb[:, dt, :], rhs, start=st, stop=sp)
                            nc.tensor.matmul(pv, wv_sb[:, dt, :], rhs, start=st, stop=sp)
                    for m in range(QH):
                        _rope(nc, ropet, qrot[m], cols, pq[m], cos_sb, sin_sb, F32, ALU)
                    _rope(nc, ropet, krot, cols, pk, cos_sb, sin_sb, F32, ALU)
                    nc.vector.tensor_copy(out=vbf[:, cols], in_=pv)

            # ---------------- attention (S^T form) + o_proj, interleaved -----
            with tc.tile_pool(name="sps", bufs=2, space="PSUM") as sps, \
                 tc.tile_pool(name="pvps", bufs=2, space="PSUM") as pvps, \
                 tc.tile_pool(name="auxps", bufs=2, space="PSUM") as auxps, \
                 tc.tile_pool(name="ops", bufs=2, space="PSUM") as ops, \
                 tc.tile_pool(name="etp", bufs=4) as etp, \
                 tc.tile_pool(name="smallp", bufs=2) as smallp, \
                 tc.tile_pool(name="osb", bufs=2) as osb:
                # v: [hd, s] -> natural [s, hd] 128-blocks via PE transpose
                for t4 in range(NT // 4):
                    vt = auxps.tile([128, 512], BF16, tag="aux")
                    for ii in range(4):
                        t = t4 * 4 + ii
                        nc.tensor.transpose(vt[:, ii * 128:(ii + 1) * 128],
                                            vbf[:, t * 128:(t + 1) * 128], idb_sb)
                    nc.vector.tensor_copy(out=vnat[:, t4 * 512:(t4 + 1) * 512], in_=vt)

                for c in range(NQ):
                    qcols = slice(c * QC, (c + 1) * QC)
                    for h in range(QH):
                        pv_ps = pvps.tile([128, QC], F32, tag="pv")
                        jmax = 4 * c + 3
                        for j in range(jmax + 1):
                            qlo = max(0, j * 128 - c * QC)
                            s_ps = sps.tile([128, QC], F32, tag="s")
                            nc.tensor.matmul(s_ps[:, qlo:QC],
                                             krot[:, j * 128:(j + 1) * 128],
                                             qrot[h][:, c * QC + qlo:(c + 1) * QC],
                                             start=True, stop=True)
                            if j >= 4 * c:          # diagonal block: causal mask
                                off = j * 128 - c * QC
                                nc.vector.tensor_tensor(out=s_ps[:, off:off + 128],
                                                        in0=s_ps[:, off:off + 128],
                                                        in1=trit_sb, op=ALU.add)
                            et = etp.tile([128, QC], BF16, tag="et")
                            nc.scalar.activation(out=et[:, qlo:QC], in_=s_ps[:, qlo:QC],
                                                 func=AF.Exp, scale=SCALE)
                            # row-sum accumulation on the Pool engine
                            if j == 0:
                                nc.gpsimd.tensor_copy(out=racc[h][:, qcols],
                                                      in_=et)
                            else:
                                nc.gpsimd.tensor_tensor(out=racc[h][:, c * QC + qlo:(c + 1) * QC],
                                                        in0=racc[h][:, c * QC + qlo:(c + 1) * QC],
                                                        in1=et[:, qlo:QC], op=ALU.add)
                            nc.tensor.matmul(pv_ps[:, qlo:QC],
                                             vnat[:, j * 128:(j + 1) * 128],
                                             et[:, qlo:QC],
                                             start=(j == 0), stop=(j == jmax))
                        # l + 1/l for the 4 q-tiles of this chunk
                        rt_ps = auxps.tile([128, 512], F32, tag="aux")
                        for i in range(4):
                            t = 4 * c + i
                            col = h * NT + t
                            nc.tensor.transpose(rt_ps[:, i * 128:(i + 1) * 128],
                                                racc[h][:, t * 128:(t + 1) * 128], idf_sb)
                            nc.vector.tensor_reduce(out=l_sb[:, col:col + 1],
                                                    in_=rt_ps[:, i * 128:(i + 1) * 128],
                                                    axis=AX.X, op=ALU.add)
                            nc.vector.reciprocal(out=linv_sb[:, col:col + 1],
                                                 in_=l_sb[:, col:col + 1])
                        # aout normalize via double transpose (q on partitions)
                        pv_sb = smallp.tile([128, QC], BF16, tag="pvsb")
                        nc.vector.tensor_copy(out=pv_sb, in_=pv_ps)
                        aT_ps = auxps.tile([128, 512], BF16, tag="aux")
                        for i in range(4):
                            nc.tensor.transpose(aT_ps[:, i * 128:(i + 1) * 128],
                                                pv_sb[:, i * 128:(i + 1) * 128], idb_sb)
                        aN_sb = smallp.tile([128, QC], BF16, tag="ansb")
                        for i in range(4):
                            col = h * NT + 4 * c + i
                            nc.scalar.activation(out=aN_sb[:, i * 128:(i + 1) * 128],
                                                 in_=aT_ps[:, i * 128:(i + 1) * 128],
                                                 func=AF.Copy,
                                                 scale=linv_sb[:, col:col + 1])
                        af_ps = auxps.tile([128, 512], BF16, tag="aux")
                        for i in range(4):
                            nc.tensor.transpose(af_ps[:, i * 128:(i + 1) * 128],
                                                aN_sb[:, i * 128:(i + 1) * 128], idb_sb)
                        nc.vector.tensor_copy(out=afin[h][:, qcols], in_=af_ps)

                    # o_proj for this chunk's 4 q-tiles (both heads ready)
                    for i in range(4):
                        t = 4 * c + i
                        o_sb = osb.tile([128, D], BF16, tag="osb")
                        for n in range(D // 512):
                            po = ops.tile([128, 512], F32, tag="po")
                            for h in range(QH):
                                nc.tensor.matmul(po, afin[h][:, t * 128:(t + 1) * 128],
                                                 wo_sb[:, h, n * 512:(n + 1) * 512],
                                                 start=(h == 0), stop=(h == QH - 1))
                            dst = o_sb[:, n * 512:(n + 1) * 512]
                            if n % 2 == 0:
                                nc.scalar.copy(out=dst, in_=po)
                            else:
                                nc.vector.tensor_copy(out=dst, in_=po)
                        nc.sync.dma_start(out=out3[:, t, :], in_=o_sb)

    nc.compile()
    return nc


def _pm(x):
    """[n*128, M] row-major -> partition-major [128, n*M]."""
    n = x.shape[0] // 128
    return np.ascontiguousarray(
        x.reshape(n, 128, x.shape[1]).transpose(1, 0, 2).reshape(128, -1))


def prep_in_maps(hidden_states, position_ids, Wq, Wk, Wv, Wo):
    import ml_dtypes
    hs = np.asarray(hidden_states, dtype=np.float32).reshape(S, D)
    hsT_pm = _pm(np.ascontiguousarray(hs.T))                       # [128, DTC*S]
    # [128, dt, S] -> quarter-major [128, sq, dt, 512] fp16
    hs_q = (hsT_pm.reshape(128, DTC, NQ, QC).transpose(0, 2, 1, 3)
            .reshape(128, -1).astype(np.float16))

    pos = np.asarray(position_ids).reshape(S).astype(np.float32)
    inv = (ROPE_BASE ** (-np.arange(0, HD, 2, dtype=np.float32) / HD))  # [64]
    ang = np.concatenate([pos[None, :] * inv[:, None]] * 2, axis=0)     # [128, S]
    cos_t = np.cos(ang).astype(np.float32)
    sin_t = np.sin(ang).astype(np.float32)
    sin_signed = np.concatenate([-sin_t[:64], sin_t[64:]], axis=0)

    k_idx = np.arange(128)[:, None]   # partition = k
    q_idx = np.arange(128)[None, :]   # col = q
    trit = np.where(q_idx >= k_idx, 0.0, NEG).astype(np.float32)
    idb = np.eye(128, dtype=ml_dtypes.bfloat16)
    idf = np.eye(128, dtype=np.float32)

    Wq = np.asarray(Wq, np.float32)
    Wk = np.asarray(Wk, np.float32)
    Wv = np.asarray(Wv, np.float32)
    Wo = np.asarray(Wo, np.float32)

    in_maps = []
    for c in range(NCORES):
        g = (c * QH) // (H // KV)          # kv head owned by this core
        wq_c = Wq[c * QH * 128:(c + 1) * QH * 128]      # [256, D]
        wk_c = Wk[g * 128:(g + 1) * 128]                # [128, D]
        wv_c = Wv[g * 128:(g + 1) * 128]                # [128, D]
        wo_c = Wo[:, c * QH * 128:(c + 1) * QH * 128]   # [D, 256]
        in_maps.append({
            "hs": hs_q,
            "wq": _pm(np.ascontiguousarray(wq_c.T)).astype(np.float16),
            "wk": _pm(np.ascontiguousarray(wk_c.T)).astype(np.float16),
            "wv": _pm(np.ascontiguousarray(wv_c.T)).astype(np.float16),
            "wo": _pm(np.ascontiguousarray(wo_c.T)).astype(ml_dtypes.bfloat16),
            "cos": cos_t,
            "sin": sin_signed,
            "trit": trit,
            "idb": idb,
            "idf": idf,
        })
    return in_maps


def combine_outputs(results):
    total = np.zeros((S, D), np.float32)
    for r in results:
        o = np.asarray(r["out"]).astype(np.float32)
        total += o.reshape(128, NT, D).transpose(1, 0, 2).reshape(S, D)
    return total[None]


def kernel(hidden_states, attention_mask, position_ids, Wq, Wk, Wv, Wo):
    from concourse import bass_utils
    if "nc" not in _CACHE:
        _CACHE["nc"] = build_nc()
    nc = _CACHE["nc"]
    in_maps = prep_in_maps(hidden_states, position_ids, Wq, Wk, Wv, Wo)
    res = bass_utils.run_bass_kernel_spmd(nc, in_maps, core_ids=list(range(NCORES)))
    return combine_outputs(res.results)


# revision 5
# speedup vs baseline: 1.5711x; 1.0156x over previous
"""LlamaAttention (B=1, S=2048, D=2048, H=16, KV=4) on 8 TRN2 NeuronCores.

Tensor-parallel over heads: core c owns q-heads [2c, 2c+1] and kv-head c//2.
Each core computes partial = attn_out_c @ Wo[:, c-slice].T over the full
sequence; the all-reduce after o_proj happens on the host (sum of partials).

v2 design (vs the fp32r baseline):
  * QKV projections run in fp16 (hs + Wq/Wk/Wv fp16): halves the input DMA
    and matches fp32r accuracy.  Quarter-sized (512-col) PSUM accumulation
    with chunked, double-buffered hs DMAs so the first matmul starts early.
  * Attention is computed TRANSPOSED: sT[k,q] = krot_j^T qrot (bf16), causal
    mask added to the diagonal block by a PE accumulate-matmul (identity
    lhsT x trit rhs), exp -> E^T tiles, PV via lhsT=vnat (natural [s,hd] V)
    streaming E^T -> aout[hd,q].  No per-block P transposes at all.
  * Softmax row sums l[q]: M=1 matmuls (ones lhsT) streaming the same E^T
    tiles accumulate [1, 512] rows in PSUM; a 2KB SBUF->SBUF DMA scatters
    the row into [128, 4] columns for the per-partition normalization scale.
    Vector/scalar/pool engines stay off the softmax critical path.
  * Normalization rides a double PE-transpose of aout (bf16, cheap): the
    intermediate [q, hd] orientation puts q on partitions, so the 1/l scale
    folds into the PSUM->SBUF copy (scalar activation, per-partition scale).
  * o_proj (bf16) is interleaved per 512-col chunk with attention; output
    partials are written bf16 ([128, t, D] per q-tile), summed on host.
"""
import math
import numpy as np

S = 2048
D = 2048
HD = 128
H = 16
KV = 4
NCORES = 8
NT = S // 128           # 16 sequence tiles
DTC = D // 128          # 16 feature chunks
NQ = 4                  # 512-col sequence quarters
QC = S // NQ            # 512
QH = H // NCORES        # 2 q-heads per core
ROPE_BASE = 10000.0
SCALE = 1.0 / math.sqrt(HD)
NEG = -1.0e9

_CACHE = {}


def _rope(nc, pool, dst, cols, src_ps, cos_sb, sin_sb, F32, ALU):
    """dst[:, cols] = src*cos + rotate_half(src)*sin  (src: psum [128, 512])."""
    w = cols.stop - cols.start
    tmp = pool.tile([128, w], F32, tag="ropetmp")
    nc.scalar.copy(out=tmp[0:64, :], in_=src_ps[64:128, :])
    nc.scalar.copy(out=tmp[64:128, :], in_=src_ps[0:64, :])
    nc.vector.tensor_tensor(out=dst[:, cols], in0=src_ps, in1=cos_sb[:, cols], op=ALU.mult)
    nc.vector.tensor_tensor(out=tmp, in0=tmp, in1=sin_sb[:, cols], op=ALU.mult)
    nc.vector.tensor_tensor(out=dst[:, cols], in0=dst[:, cols], in1=tmp, op=ALU.add)


def build_nc():
    import concourse.bacc as bacc
    import concourse.tile as tile
    from concourse import mybir

    F32 = mybir.dt.float32
    F16 = mybir.dt.float16
    BF16 = mybir.dt.bfloat16
    AF = mybir.ActivationFunctionType
    ALU = mybir.AluOpType

    nc = bacc.Bacc("TRN2", target_bir_lowering=False, debug=False)
    hs_d = nc.dram_tensor("hs", [128, NQ * DTC * QC], F16, kind="ExternalInput").ap()
    wq_d = nc.dram_tensor("wq", [128, DTC * QH * 128], F16, kind="ExternalInput").ap()
    wk_d = nc.dram_tensor("wk", [128, DTC * 128], F16, kind="ExternalInput").ap()
    wv_d = nc.dram_tensor("wv", [128, DTC * 128], F16, kind="ExternalInput").ap()
    wo_d = nc.dram_tensor("wo", [128, QH * D], BF16, kind="ExternalInput").ap()
    cos_d = nc.dram_tensor("cos", [128, S], F16, kind="ExternalInput").ap()
    sin_d = nc.dram_tensor("sin", [128, S], F16, kind="ExternalInput").ap()
    trit_d = nc.dram_tensor("trit", [128, 128], BF16, kind="ExternalInput").ap()
    idb_d = nc.dram_tensor("idb", [128, 128], BF16, kind="ExternalInput").ap()
    out_d = nc.dram_tensor("out", [128, NT * D], BF16, kind="ExternalOutput").ap()
    lscr_d = nc.dram_tensor("lscr", [QH * NQ, QC], F32, kind="Internal").ap()

    hs4 = hs_d.rearrange("p (q t s) -> p q t s", q=NQ, t=DTC)
    wq3 = wq_d.rearrange("p (t m) -> p t m", t=DTC)
    wk3 = wk_d.rearrange("p (t m) -> p t m", t=DTC)
    wv3 = wv_d.rearrange("p (t m) -> p t m", t=DTC)
    out3 = out_d.rearrange("p (t d) -> p t d", t=NT)

    with tile.TileContext(nc) as tc:
        with tc.tile_pool(name="consts", bufs=1) as consts, \
             tc.tile_pool(name="persist", bufs=1) as persist:
            cos_sb = consts.tile([128, S], F16)
            sin_sb = consts.tile([128, S], F16)
            trit_sb = consts.tile([128, 128], BF16)
            idb_sb = consts.tile([128, 128], BF16)
            ones_sb = consts.tile([128, 1], BF16)
            wq_sb = consts.tile([128, DTC, QH * 128], F16)
            wk_sb = consts.tile([128, DTC, 128], F16)
            wv_sb = consts.tile([128, DTC, 128], F16)
            wo_sb = consts.tile([128, QH, D], BF16)

            # critical-path first: weights in 4-dt-group slices, then rope
            # tables; attention-phase consts (trit/idb/wo) last.
            for g4 in range(4):
                ts = slice(g4 * 4, (g4 + 1) * 4)
                nc.sync.dma_start(out=wq_sb[:, ts, :], in_=wq3[:, ts, :])
                nc.sync.dma_start(out=wk_sb[:, ts, :], in_=wk3[:, ts, :])
                nc.sync.dma_start(out=wv_sb[:, ts, :], in_=wv3[:, ts, :])
            nc.sync.dma_start(out=cos_sb, in_=cos_d)
            nc.sync.dma_start(out=sin_sb, in_=sin_d)
            nc.sync.dma_start(out=trit_sb, in_=trit_d)
            nc.sync.dma_start(out=idb_sb, in_=idb_d)
            nc.gpsimd.memset(ones_sb, 1.0)
            nc.sync.dma_start(out=wo_sb, in_=wo_d.rearrange("p (h m) -> p h m", h=QH))

            qrot = [persist.tile([128, S], BF16, tag=f"qrot{h}", name=f"qrot{h}")
                    for h in range(QH)]
            krot = persist.tile([128, S], BF16, tag="krot")
            vbf = persist.tile([128, S], BF16, tag="vbf")
            vnat = persist.tile([128, NT * 128], BF16, tag="vnat")
            afin = [persist.tile([128, S], BF16, tag=f"afin{h}", name=f"afin{h}")
                    for h in range(QH)]
            linv_sb = persist.tile([128, QH * NQ, 4], F32, tag="linv")
            lcol_sb = persist.tile([128, QH * NQ, 4], F32, tag="lcol")
            lrow_sb = persist.tile([1, QH * NQ, QC], F32, tag="lrow")

            # ---------------- QKV projections (+RoPE), 512-col quarters ------
            with tc.tile_pool(name="hsp", bufs=3) as hsp, \
                 tc.tile_pool(name="ropet", bufs=2) as ropet, \
                 tc.tile_pool(name="qkvps", bufs=2, space="PSUM") as qkvps:
                for sq in range(NQ):
                    cols = slice(sq * QC, (sq + 1) * QC)
                    pq = [qkvps.tile([128, QC], F32, tag=f"pq{m}", name=f"pq{m}")
                          for m in range(QH)]
                    pk = qkvps.tile([128, QC], F32, tag="pk")
                    pv = qkvps.tile([128, QC], F32, tag="pv")
                    for g4 in range(4):
                        hst = hsp.tile([128, 4, QC], F16, tag="hst")
                        nc.sync.dma_start(out=hst,
                                          in_=hs4[:, sq, g4 * 4:(g4 + 1) * 4, :])
                        for t4 in range(4):
                            dt = g4 * 4 + t4
                            st = dt == 0
                            sp = dt == DTC - 1
                            rhs = hst[:, t4, :]
                            for m in range(QH):
                                nc.tensor.matmul(pq[m], wq_sb[:, dt, m * 128:(m + 1) * 128],
                                                 rhs, start=st, stop=sp)
                            nc.tensor.matmul(pk, wk_sb[:, dt, :], rhs, start=st, stop=sp)
                            nc.tensor.matmul(pv, wv_sb[:, dt, :], rhs, start=st, stop=sp)
                    for m in range(QH):
                        _rope(nc, ropet, qrot[m], cols, pq[m], cos_sb, sin_sb, F32, ALU)
                    _rope(nc, ropet, krot, cols, pk, cos_sb, sin_sb, F32, ALU)
                    nc.vector.tensor_copy(out=vbf[:, cols], in_=pv)

            # ---------------- attention (S^T form) + o_proj, interleaved -----
            with tc.tile_pool(name="sps", bufs=2, space="PSUM") as sps, \
                 tc.tile_pool(name="pvps", bufs=1, space="PSUM") as pvps, \
                 tc.tile_pool(name="auxps", bufs=1, space="PSUM") as auxps, \
                 tc.tile_pool(name="lps", bufs=2, space="PSUM") as lps, \
                 tc.tile_pool(name="ops", bufs=2, space="PSUM") as ops, \
                 tc.tile_pool(name="etp", bufs=4) as etp, \
                 tc.tile_pool(name="smallp", bufs=2) as smallp, \
                 tc.tile_pool(name="osb", bufs=2) as osb:
                # v: [hd, s] -> natural [s, hd] 128-blocks via PE transpose
                for t4 in range(NT // 4):
                    vt = auxps.tile([128, 512], BF16, tag="aux")
                    for ii in range(4):
                        t = t4 * 4 + ii
                        nc.tensor.transpose(vt[:, ii * 128:(ii + 1) * 128],
                                            vbf[:, t * 128:(t + 1) * 128], idb_sb)
                    nc.vector.tensor_copy(out=vnat[:, t4 * 512:(t4 + 1) * 512], in_=vt)

                for c in range(NQ):
                    qcols = slice(c * QC, (c + 1) * QC)
                    for h in range(QH):
                        hc = h * NQ + c
                        pv_ps = pvps.tile([128, QC], F32, tag="pv")
                        l_ps = lps.tile([1, QC], F32, tag="lrow")
                        jmax = 4 * c + 3
                        for j in range(jmax + 1):
                            qlo = max(0, j * 128 - c * QC)
                            s_ps = sps.tile([128, QC], F32, tag="s")
                            diag = j >= 4 * c
                            nc.tensor.matmul(s_ps[:, qlo:QC],
                                             krot[:, j * 128:(j + 1) * 128],
                                             qrot[h][:, c * QC + qlo:(c + 1) * QC],
                                             start=True, stop=not diag)
                            if diag:   # causal mask: accumulate trit via PE
                                nc.tensor.matmul(s_ps[:, qlo:qlo + 128],
                                                 idb_sb, trit_sb,
                                                 start=False, stop=True)
                            et = etp.tile([128, QC], BF16, tag="et")
                            nc.scalar.activation(out=et[:, qlo:QC], in_=s_ps[:, qlo:QC],
                                                 func=AF.Exp, scale=SCALE)
                            nc.tensor.matmul(pv_ps[:, qlo:QC],
                                             vnat[:, j * 128:(j + 1) * 128],
                                             et[:, qlo:QC],
                                             start=(j == 0), stop=(j == jmax))
                            nc.tensor.matmul(l_ps[:, qlo:QC],
                                             ones_sb, et[:, qlo:QC],
                                             start=(j == 0), stop=(j == jmax))
                        # l row -> DRAM bounce -> [128, 4] columns -> 1/l
                        nc.vector.tensor_copy(out=lrow_sb[:, hc, :], in_=l_ps)
                        nc.sync.dma_start(out=lscr_d[hc:hc + 1, :],
                                          in_=lrow_sb[:, hc, :])
                        nc.sync.dma_start(
                            out=lcol_sb[:, hc, :],
                            in_=lscr_d[hc:hc + 1, :].rearrange("o (i p) -> (o p) i", p=128))
                        nc.vector.reciprocal(out=linv_sb[:, hc, :], in_=lcol_sb[:, hc, :])
                        # aout normalize via double transpose (q on partitions)
                        pv_sb = smallp.tile([128, QC], BF16, tag="pvsb")
                        nc.vector.tensor_copy(out=pv_sb, in_=pv_ps)
                        aT_ps = auxps.tile([128, 512], BF16, tag="aux")
                        for i in range(4):
                            nc.tensor.transpose(aT_ps[:, i * 128:(i + 1) * 128],
                                                pv_sb[:, i * 128:(i + 1) * 128], idb_sb)
                        aN_sb = smallp.tile([128, QC], BF16, tag="ansb")
                        for i in range(4):
                            nc.scalar.activation(out=aN_sb[:, i * 128:(i + 1) * 128],
                                                 in_=aT_ps[:, i * 128:(i + 1) * 128],
                                                 func=AF.Copy,
                                                 scale=linv_sb[:, hc, i:i + 1])
                        af_ps = auxps.tile([128, 512], BF16, tag="aux")
                        for i in range(4):
                            nc.tensor.transpose(af_ps[:, i * 128:(i + 1) * 128],
                                                aN_sb[:, i * 128:(i + 1) * 128], idb_sb)
                        nc.vector.tensor_copy(out=afin[h][:, qcols], in_=af_ps)

                    # o_proj for this chunk's 4 q-tiles (both heads ready)
                    for i in range(4):
                        t = 4 * c + i
                        o_sb = osb.tile([128, D], BF16, tag="osb")
                        for n in range(D // 512):
                            po = ops.tile([128, 512], F32, tag="po")
                            for h in range(QH):
                                nc.tensor.matmul(po, afin[h][:, t * 128:(t + 1) * 128],
                                                 wo_sb[:, h, n * 512:(n + 1) * 512],
                                                 start=(h == 0), stop=(h == QH - 1))
                            dst = o_sb[:, n * 512:(n + 1) * 512]
                            if n % 2 == 0:
                                nc.scalar.copy(out=dst, in_=po)
                            else:
                                nc.vector.tensor_copy(out=dst, in_=po)
                        nc.sync.dma_start(out=out3[:, t, :], in_=o_sb)

    nc.compile()
    return nc


def _pm(x):
    """[n*128, M] row-major -> partition-major [128, n*M]."""
    n = x.shape[0] // 128
    return np.ascontiguousarray(
        x.reshape(n, 128, x.shape[1]).transpose(1, 0, 2).reshape(128, -1))


def prep_in_maps(hidden_states, position_ids, Wq, Wk, Wv, Wo):
    import ml_dtypes
    hs = np.asarray(hidden_states, dtype=np.float32).reshape(S, D)
    hsT_pm = _pm(np.ascontiguousarray(hs.T))                       # [128, DTC*S]
    # [128, dt, S] -> quarter-major [128, sq, dt, 512] fp16
    hs_q = (hsT_pm.reshape(128, DTC, NQ, QC).transpose(0, 2, 1, 3)
            .reshape(128, -1).astype(np.float16))

    pos = np.asarray(position_ids).reshape(S).astype(np.float32)
    inv = (ROPE_BASE ** (-np.arange(0, HD, 2, dtype=np.float32) / HD))  # [64]
    ang = np.concatenate([pos[None, :] * inv[:, None]] * 2, axis=0)     # [128, S]
    cos_t = np.cos(ang).astype(np.float16)
    sin_t = np.sin(ang)
    sin_signed = np.concatenate([-sin_t[:64], sin_t[64:]], axis=0).astype(np.float16)

    k_idx = np.arange(128)[:, None]   # partition = k
    q_idx = np.arange(128)[None, :]   # col = q
    trit = np.where(q_idx >= k_idx, 0.0, NEG).astype(ml_dtypes.bfloat16)
    idb = np.eye(128, dtype=ml_dtypes.bfloat16)

    Wq = np.asarray(Wq, np.float32)
    Wk = np.asarray(Wk, np.float32)
    Wv = np.asarray(Wv, np.float32)
    Wo = np.asarray(Wo, np.float32)

    in_maps = []
    for c in range(NCORES):
        g = (c * QH) // (H // KV)          # kv head owned by this core
        wq_c = Wq[c * QH * 128:(c + 1) * QH * 128]      # [256, D]
        wk_c = Wk[g * 128:(g + 1) * 128]                # [128, D]
        wv_c = Wv[g * 128:(g + 1) * 128]                # [128, D]
        wo_c = Wo[:, c * QH * 128:(c + 1) * QH * 128]   # [D, 256]
        in_maps.append({
            "hs": hs_q,
            "wq": _pm(np.ascontiguousarray(wq_c.T)).astype(np.float16),
            "wk": _pm(np.ascontiguousarray(wk_c.T)).astype(np.float16),
            "wv": _pm(np.ascontiguousarray(wv_c.T)).astype(np.float16),
            "wo": _pm(np.ascontiguousarray(wo_c.T)).astype(ml_dtypes.bfloat16),
            "cos": cos_t,
            "sin": sin_signed,
            "trit": trit,
            "idb": idb,
        })
    return in_maps


def combine_outputs(results):
    total = np.zeros((S, D), np.float32)
    for r in results:
        o = np.asarray(r["out"]).astype(np.float32)
        total += o.reshape(128, NT, D).transpose(1, 0, 2).reshape(S, D)
    return total[None]


def kernel(hidden_states, attention_mask, position_ids, Wq, Wk, Wv, Wo):
    from concourse import bass_utils
    if "nc" not in _CACHE:
        _CACHE["nc"] = build_nc()
    nc = _CACHE["nc"]
    in_maps = prep_in_maps(hidden_states, position_ids, Wq, Wk, Wv, Wo)
    res = bass_utils.run_bass_kernel_spmd(nc, in_maps, core_ids=list(range(NCORES)))
    return combine_outputs(res.results)
